# revision 3
# baseline (speedup 1.0000x reference)
"""AnomalyTransformer forward pass on 8 Trainium2 NeuronCores.

Data-parallel over batch: each core processes 32 of the 256 batch elements
through the full 3-layer transformer. All compute in bf16 on the
TensorEngine with fp32 PSUM accumulation.

Layout strategy: the residual stream h is kept feature-major ([D, tokens],
D split over 4 partition-tiles of 128). Attention uses the scoresT
orientation (scoresT = khT.T @ qhT, [l_k, l_q]) so that softmax
normalization folds into the attention-value matmul via an appended
ones-column on V (the 65th column accumulates sum(exp)); the per-token
reciprocal is then a per-partition scalar. One [100, 512] PE-transpose per
batch element brings the attention output back to feature-major.

The sigma/prior branch of the reference is dead code (never feeds the
output) and is skipped. Biases in the reference are all zeros and are
skipped.
"""

import sys
import os
for _p in ("/opt/trn_rl_repo", "/root/.axon_site/_ro/trn_rl_repo"):
    if os.path.isdir(_p) and _p not in sys.path:
        sys.path.insert(0, _p)

import math
import numpy as np
import ml_dtypes

import concourse.bass as bass
import concourse.tile as tile
from concourse import mybir
from concourse.bass_utils import run_bass_kernel_spmd
from concourse.masks import make_identity
from contextlib import ExitStack

BF16 = mybir.dt.bfloat16
F32 = mybir.dt.float32
AF = mybir.ActivationFunctionType
OP = mybir.AluOpType

# model dims
B, L, C, D, H, NL, DFF = 256, 100, 55, 512, 8, 3, 64
DK = D // H                      # 64
NCORES = 8
BL = B // NCORES                 # 32 batches per core
TOK = BL * L                     # 3200 tokens per core
TCH = 400                        # token chunk (4 batches)
NT = TOK // TCH                  # 8 chunks
CB = TCH // L                    # 4 batches per chunk
KT = D // 128                    # 4 contraction tiles
C3 = 3 * C                       # 165 unfolded conv rows


def _legalize_waits(nc, maxw=1):
    """This container's walrus rejects >1 sync-wait per instruction; move
    extra waits onto preceding same-engine NOPs."""
    cnt = [0]
    for f in nc.m.functions:
        for blk in f.blocks:
            newlist = []
            changed = False
            for ins in blk.instructions:
                si = getattr(ins, "sync_info", None)
                if si is not None and si.on_wait and len(si.on_wait) > maxw:
                    waits = list(si.on_wait)
                    extra, keep = waits[:-maxw], waits[-maxw:]
                    for i in range(0, len(extra), maxw):
                        cnt[0] += 1
                        nop = mybir.InstNoOp(
                            name=f"I-ws-{cnt[0]}", ins=[], outs=[], engine=ins.engine
                        )
                        nop.sync_info = mybir.SyncInfo(
                            on_wait=extra[i:i + maxw], on_update=[]
                        )
                        newlist.append(nop)
                    ins.sync_info = mybir.SyncInfo(
                        on_wait=keep, on_update=list(si.on_update)
                    )
                    changed = True
                newlist.append(ins)
            if changed:
                blk.instructions = newlist
    return nc


def _offset_ap(ap, extra_offset, dims):
    """AP at ap.offset + extra_offset (elements) with free dims `dims`
    ([[step, count], ...]), keeping ap's partition dim."""
    return bass.AP(tensor=ap.tensor, offset=ap.offset + extra_offset,
                   ap=[list(ap.ap[0])] + [list(d) for d in dims])


def build_nc():
    nc = bass.Bass()

    # ---- DRAM parameters (host-prepped, bf16) ----
    xcat0 = nc.declare_dram_parameter("xcat0", [128, TOK], BF16, isOutput=False)
    xcat1 = nc.declare_dram_parameter("xcat1", [C3 - 128, TOK], BF16, isOutput=False)
    wemb0 = nc.declare_dram_parameter("wemb0", [128, D], BF16, isOutput=False)
    wemb1 = nc.declare_dram_parameter("wemb1", [C3 - 128, D], BF16, isOutput=False)
    pe_d = nc.declare_dram_parameter("pe", [128, KT, L], BF16, isOutput=False)
    wq_d = nc.declare_dram_parameter("wq", [128, NL, KT, D], BF16, isOutput=False)
    wk_d = nc.declare_dram_parameter("wk", [128, NL, KT, D], BF16, isOutput=False)
    wv_d = nc.declare_dram_parameter("wv", [128, NL, KT, D], BF16, isOutput=False)
    wo_d = nc.declare_dram_parameter("wo", [128, NL, KT, D], BF16, isOutput=False)
    w1_d = nc.declare_dram_parameter("w1", [128, NL, KT, DFF], BF16, isOutput=False)
    w2_d = nc.declare_dram_parameter("w2", [DFF, NL, KT, 128], BF16, isOutput=False)
    wout_d = nc.declare_dram_parameter("wout", [128, KT, C], BF16, isOutput=False)
    out_d = nc.declare_dram_parameter("out", [C, TOK], F32, isOutput=True)

    with tile.TileContext(nc) as tc, ExitStack() as stk:
        tc.race_detector_enabled = False
        singles = stk.enter_context(tc.tile_pool(name="singles", bufs=1))
        # sbuf working pools
        expp = stk.enter_context(tc.tile_pool(name="expp", bufs=3))
        op_ = stk.enter_context(tc.tile_pool(name="op", bufs=3))
        rp = stk.enter_context(tc.tile_pool(name="rp", bufs=3))
        otp = stk.enter_context(tc.tile_pool(name="otp", bufs=2))
        vp = stk.enter_context(tc.tile_pool(name="vp", bufs=2))
        xp = stk.enter_context(tc.tile_pool(name="xp", bufs=2))
        yp = stk.enter_context(tc.tile_pool(name="yp", bufs=2))
        outp = stk.enter_context(tc.tile_pool(name="outp", bufs=2))
        # psum pools (8 banks total)
        pp = stk.enter_context(tc.tile_pool(name="pp", bufs=2, space="PSUM"))
        scp = stk.enter_context(tc.tile_pool(name="scp", bufs=2, space="PSUM"))
        oup = stk.enter_context(tc.tile_pool(name="oup", bufs=2, space="PSUM"))
        tpp = stk.enter_context(tc.tile_pool(name="tpp", bufs=2, space="PSUM"))

        # ---- persistent SBUF ----
        wemb0_sb = singles.tile([128, D], BF16)
        wemb1_sb = singles.tile([C3 - 128, D], BF16)
        pe_sb = singles.tile([128, KT, L], BF16)
        wq_sb = singles.tile([128, NL, KT, D], BF16)
        wk_sb = singles.tile([128, NL, KT, D], BF16)
        wv_sb = singles.tile([128, NL, KT, D], BF16)
        wo_sb = singles.tile([128, NL, KT, D], BF16)
        w1_sb = singles.tile([128, NL, KT, DFF], BF16)
        w2_sb = singles.tile([DFF, NL, KT, 128], BF16)
        wout_sb = singles.tile([128, KT, C], BF16)
        ident = singles.tile([128, 128], BF16)
        h_sb = [singles.tile([128, TOK], BF16, name=f"h{k}") for k in range(KT)]
        q_sb = [singles.tile([128, TOK], BF16, name=f"q{k}") for k in range(KT)]
        k_sb = [singles.tile([128, TOK], BF16, name=f"k{k}") for k in range(KT)]

        for dst, src in ((wemb0_sb, wemb0),
                         (wemb1_sb, wemb1), (pe_sb, pe_d), (wq_sb, wq_d),
                         (wk_sb, wk_d), (wv_sb, wv_d), (wo_sb, wo_d),
                         (w1_sb, w1_d), (w2_sb, w2_d), (wout_sb, wout_d)):
            nc.sync.dma_start(out=dst[:], in_=src[:])

        make_identity(nc, ident)

        # ---- token embedding: circular conv as matmul, + positional emb ----
        for t in range(NT):
            tsl = slice(t * TCH, (t + 1) * TCH)
            xc0 = xp.tile([128, TCH], BF16, tag="xc0")
            xc1 = xp.tile([C3 - 128, TCH], BF16, tag="xc1")
            nc.sync.dma_start(out=xc0[:], in_=xcat0[:, tsl])
            nc.sync.dma_start(out=xc1[:], in_=xcat1[:, tsl])
            for m in range(KT):
                ps = pp.tile([128, 512], F32, tag="pp")
                nc.tensor.matmul(ps[:, :TCH], wemb0_sb[:, m * 128:(m + 1) * 128],
                                 xc0[:], start=True, stop=False)
                nc.tensor.matmul(ps[:, :TCH], wemb1_sb[:, m * 128:(m + 1) * 128],
                                 xc1[:], start=False, stop=True)
                pe_b = _offset_ap(pe_sb[:, m, :], 0, [[0, CB], [1, L]])
                nc.vector.tensor_tensor(
                    h_sb[m][:, tsl].rearrange("p (b x) -> p b x", x=L),
                    ps[:, :TCH].rearrange("p (b x) -> p b x", x=L),
                    pe_b, op=OP.add)

        # ---- transformer layers ----
        for l in range(NL):
            # Q and K projections, feature-major [D, TOK]
            for t in range(NT):
                tsl = slice(t * TCH, (t + 1) * TCH)
                for m in range(KT):
                    msl = slice(m * 128, (m + 1) * 128)
                    ps = pp.tile([128, 512], F32, tag="pp")
                    for k in range(KT):
                        nc.tensor.matmul(ps[:, :TCH], wq_sb[:, l, k, msl],
                                         h_sb[k][:, tsl],
                                         start=(k == 0), stop=(k == KT - 1))
                    nc.vector.tensor_copy(q_sb[m][:, tsl], ps[:, :TCH])
                for m in range(KT):
                    msl = slice(m * 128, (m + 1) * 128)
                    ps = pp.tile([128, 512], F32, tag="pp")
                    for k in range(KT):
                        nc.tensor.matmul(ps[:, :TCH], wk_sb[:, l, k, msl],
                                         h_sb[k][:, tsl],
                                         start=(k == 0), stop=(k == KT - 1))
                    nc.vector.tensor_copy(k_sb[m][:, tsl], ps[:, :TCH])

            # attention, per 4-batch chunk
            for g in range(NT):
                gsl = slice(g * TCH, (g + 1) * TCH)
                # V projection, token-major per batch [L, D] (65-stride heads,
                # col 65h+64 = 1 for the softmax-sum trick)
                v_t = vp.tile([128, CB, 8 * 65], BF16, tag="v")
                nc.vector.memset(
                    v_t[:L, :, :].rearrange("p b (h x) -> p b h x", x=65)[:, :, :, 64:65],
                    1.0)
                for bi in range(CB):
                    b = g * CB + bi
                    bsl = slice(b * L, (b + 1) * L)
                    ps = pp.tile([128, 512], F32, tag="pp")
                    for k in range(KT):
                        nc.tensor.matmul(ps[:L, :], h_sb[k][:, bsl], wv_sb[:, l, k, :],
                                         start=(k == 0), stop=(k == KT - 1))
                    nc.vector.tensor_copy(
                        v_t[:L, bi, :].rearrange("p (h x) -> p h x", x=65)[:, :, :64],
                        ps[:L, :].rearrange("p (h x) -> p h x", x=64))
                ot_t = [otp.tile([128, TCH], BF16, name=f"ot{m}", tag=f"ot{m}") for m in range(KT)]
                for bi in range(CB):
                    b = g * CB + bi
                    bsl = slice(b * L, (b + 1) * L)
                    # scoresT for 8 heads: even heads -> scA, odd -> scB
                    # (different PE row groups must write different PSUM banks)
                    scA = scp.tile([128, 512], F32, tag="sc")
                    scB = scp.tile([128, 512], F32, tag="sc")
                    for hh in range(8):
                        kt_i, base = divmod(hh * DK, 128)
                        sc = scA if hh % 2 == 0 else scB
                        col = (hh // 2) * 128
                        nc.tensor.matmul(sc[:L, col:col + L],
                                         k_sb[kt_i][base:base + DK, bsl],
                                         q_sb[kt_i][base:base + DK, bsl],
                                         start=True, stop=True)
                    exp_t = expp.tile([128, 8 * L], BF16, tag="exp")
                    # exp; head hh lands at exp_t cols hh*L
                    nc.scalar.activation(
                        exp_t[:L, :].rearrange("p (h x) -> p h x", x=2 * L)[:, :, :L],
                        scA[:L, :].rearrange("p (h x) -> p h x", x=128)[:, :, :L],
                        AF.Exp)
                    nc.scalar.activation(
                        _offset_ap(exp_t[:L, :], L, [[2 * L, 4], [1, L]]),
                        scB[:L, :].rearrange("p (h x) -> p h x", x=128)[:, :, :L],
                        AF.Exp)
                    # oU = expST.T @ [v | 1]  (token-major, col 64 = sum(exp))
                    ouA = oup.tile([128, 512], F32, tag="ou")
                    ouB = oup.tile([128, 512], F32, tag="ou")
                    for hh in range(8):
                        ou = ouA if hh < 4 else ouB
                        col = (hh % 4) * 128
                        nc.tensor.matmul(ou[:L, col:col + 65],
                                         exp_t[:L, hh * L:(hh + 1) * L],
                                         v_t[:L, bi, hh * 65:(hh + 1) * 65],
                                         start=True, stop=True)
                    r_t = rp.tile([128, 8], F32, tag="r")
                    o_t = op_.tile([128, D], BF16, tag="o")
                    for i, ou in enumerate((ouA, ouB)):
                        nc.vector.reciprocal(
                            r_t[:L, i * 4:(i + 1) * 4],
                            ou[:L, :].rearrange("p (h x) -> p h x", x=128)[:, :, 64:65])
                        nc.vector.tensor_tensor(
                            o_t[:L, i * 256:(i + 1) * 256].rearrange(
                                "p (h x) -> p h x", x=64),
                            ou[:L, :].rearrange("p (h x) -> p h x", x=128)[:, :, :64],
                            r_t[:L, i * 4:(i + 1) * 4].rearrange(
                                "p (h x) -> p h x", x=1).broadcast_to([L, 4, 64]),
                            op=OP.mult)
                    # transpose o back to feature-major
                    for m in range(KT):
                        tp = tpp.tile([128, 128], BF16, tag="tp")
                        nc.tensor.transpose(tp[:, :L],
                                            o_t[:L, m * 128:(m + 1) * 128],
                                            ident[:L, :L])
                        nc.scalar.copy(ot_t[m][:, bi * L:(bi + 1) * L], tp[:, :L])
                # Wo projection + residual (identity matmul accumulates h)
                for m in range(KT):
                    msl = slice(m * 128, (m + 1) * 128)
                    ps = pp.tile([128, 512], F32, tag="pp")
                    for k in range(KT):
                        nc.tensor.matmul(ps[:, :TCH], wo_sb[:, l, k, msl], ot_t[k][:],
                                         start=(k == 0), stop=False)
                    nc.tensor.matmul(ps[:, :TCH], ident[:], h_sb[m][:, gsl],
                                     start=False, stop=True)
                    nc.scalar.copy(h_sb[m][:, gsl], ps[:, :TCH])
                # FFN + residual
                ps1 = pp.tile([128, 512], F32, tag="pp")
                for k in range(KT):
                    nc.tensor.matmul(ps1[:DFF, :TCH], w1_sb[:, l, k, :],
                                     h_sb[k][:, gsl],
                                     start=(k == 0), stop=(k == KT - 1))
                y_t = yp.tile([DFF, TCH], BF16, tag="y")
                nc.scalar.activation(y_t[:, :], ps1[:DFF, :TCH], AF.Gelu)
                for m in range(KT):
                    ps2 = pp.tile([128, 512], F32, tag="pp")
                    nc.tensor.matmul(ps2[:, :TCH], w2_sb[:, l, m, :], y_t[:, :],
                                     start=True, stop=False)
                    nc.tensor.matmul(ps2[:, :TCH], ident[:], h_sb[m][:, gsl],
                                     start=False, stop=True)
                    nc.scalar.copy(h_sb[m][:, gsl], ps2[:, :TCH])

        # ---- output projection [C, TOK] ----
        for t in range(NT):
            tsl = slice(t * TCH, (t + 1) * TCH)
            ps = pp.tile([128, 512], F32, tag="pp")
            for k in range(KT):
                nc.tensor.matmul(ps[:C, :TCH], wout_sb[:, k, :], h_sb[k][:, tsl],
                                 start=(k == 0), stop=(k == KT - 1))
            o_f = outp.tile([128, TCH], F32, tag="outc")
            nc.vector.tensor_copy(o_f[:C, :], ps[:C, :TCH])
            nc.sync.dma_start(out=out_d[:, tsl], in_=o_f[:C, :])

    return _legalize_waits(nc)


def _bf(a):
    return np.ascontiguousarray(a).astype(ml_dtypes.bfloat16)


def prep_weights(tok_w, pe, Wq, Wk, Wv, Wo, W1, W2, proj_w):
    """Host-side weight reorganization (shared across cores)."""
    scale = 1.0 / math.sqrt(DK)
    # conv unfold: W_unf[55d + c, o] = tok_w[o, c, d]
    wemb = np.ascontiguousarray(np.transpose(tok_w, (2, 1, 0))).reshape(C3, D)
    # projection weights as lhsT tiles: w[p, l, k, j] = W[l, j, 128k + p]
    def proj_lhsT(W):  # [NL, D_out, D_in] -> [128, NL, KT, D_out]
        return np.ascontiguousarray(
            np.transpose(W, (2, 0, 1)).reshape(KT, 128, NL, W.shape[1])
            .transpose(1, 2, 0, 3))
    m = {
        "wemb0": _bf(wemb[:128]), "wemb1": _bf(wemb[128:]),
        "pe": _bf(np.ascontiguousarray(pe.T).reshape(KT, 128, L)
                  .transpose(1, 0, 2)),
        "wq": _bf(proj_lhsT(Wq * scale)),
        "wk": _bf(proj_lhsT(Wk)),
        "wv": _bf(proj_lhsT(Wv)),
        "wo": _bf(proj_lhsT(Wo)),
        "w1": _bf(proj_lhsT(W1)),
        # w2[p, l, m, j] = W2[l, 128m + j, p]   (p over DFF=64)
        "w2": _bf(np.transpose(W2, (2, 0, 1)).reshape(DFF, NL, KT, 128)),
        # wout[p, k, j] = proj_w[j, 128k + p]
        "wout": _bf(np.ascontiguousarray(proj_w.T).reshape(KT, 128, C)
                    .transpose(1, 0, 2)),
    }
    return m


def prep_xcat(xs):
    """Per-core input: xs [BL, L, C] -> circular-unfolded feature-major
    [165, BL*L], split into [128, .] + [37, .]."""
    xt = np.ascontiguousarray(np.transpose(xs, (2, 0, 1)))    # [C, BL, L]
    rows = [np.roll(xt, 1 - d, axis=2) for d in range(3)]     # x[t+d-1]
    xcat = np.concatenate(rows, axis=0).reshape(C3, TOK)
    return _bf(xcat[:128]), _bf(xcat[128:])


_NC_CACHE = {}


def get_nc():
    if "nc" not in _NC_CACHE:
        _NC_CACHE["nc"] = build_nc()
    return _NC_CACHE["nc"]


def make_in_maps(inputs):
    x = np.asarray(inputs["x"], np.float32)
    wm = prep_weights(np.asarray(inputs["tok_w"], np.float32),
                      np.asarray(inputs["pe"], np.float32),
                      np.asarray(inputs["Wq"], np.float32),
                      np.asarray(inputs["Wk"], np.float32),
                      np.asarray(inputs["Wv"], np.float32),
                      np.asarray(inputs["Wo"], np.float32),
                      np.asarray(inputs["W1"], np.float32),
                      np.asarray(inputs["W2"], np.float32),
                      np.asarray(inputs["proj_w"], np.float32))
    in_maps = []
    for c in range(NCORES):
        x0, x1 = prep_xcat(x[c * BL:(c + 1) * BL])
        in_maps.append({**wm, "xcat0": x0, "xcat1": x1})
    return in_maps


def assemble_out(results):
    # per-core out [C, TOK] feature-major -> [B, L, C]
    outs = [np.asarray(r["out"], np.float32).reshape(C, BL, L).transpose(1, 2, 0)
            for r in results]
    return np.concatenate(outs, axis=0)


def kernel(**inputs) -> np.ndarray:
    nc = get_nc()
    in_maps = make_in_maps(inputs)
    res = run_bass_kernel_spmd(nc, in_maps, core_ids=list(range(NCORES)))
    return assemble_out(res.results)


# revision 5
# speedup vs baseline: 607.7533x; 607.7533x over previous
"""AnomalyTransformer forward pass on 8 Trainium2 NeuronCores.

Data-parallel over batch: each core processes 32 of the 256 batch elements
through the full 3-layer transformer.

Precision strategy: the residual stream and all projections that feed the
attention logits run in float32r (TF32-like TensorEngine mode, full
throughput at moving-dim >= 256); the value/attention-output path runs in
bf16. This keeps worst-case relative error ~5e-3 (softmax logits reach
+-38 in layer 3, so bf16 q/k rounding would be amplified to ~1.5e-2).

Layout strategy: the residual stream h is feature-major ([D, tokens], D
split over 4 partition-tiles of 128). Attention uses the scoresT
orientation (scoresT = khT.T @ qhT -> [l_k, l_q]) so softmax normalization
folds into the attention-value matmul via an appended ones-column on V
(column 64 of each head's 65-wide slot accumulates sum(exp)); the
per-token reciprocal is then a per-partition scalar multiply. One
[100, 512] PE-transpose per batch element brings the attention output back
to feature-major for the Wo projection. Residual adds are folded into the
Wo/W2 PSUM accumulation as identity matmuls.

The sigma/prior branch of the reference is dead code (never feeds the
output) and is skipped. Biases in the reference are all zeros and are
skipped.
"""

import sys
import os
for _p in ("/opt/trn_rl_repo", "/root/.axon_site/_ro/trn_rl_repo"):
    if os.path.isdir(_p) and _p not in sys.path:
        sys.path.insert(0, _p)

import math
import numpy as np
import ml_dtypes

import concourse.bass as bass
import concourse.tile as tile
from concourse import mybir
from concourse.bass_utils import run_bass_kernel_spmd
from contextlib import ExitStack

BF16 = mybir.dt.bfloat16
F32 = mybir.dt.float32
F32R = mybir.dt.float32r
AF = mybir.ActivationFunctionType
OP = mybir.AluOpType

# model dims
B, L, C, D, H, NL, DFF = 256, 100, 55, 512, 8, 3, 64
DK = D // H                      # 64
NCORES = 8
BL = B // NCORES                 # 32 batches per core
TOK = BL * L                     # 3200 tokens per core
TCH = 400                        # token chunk (4 batches)
NT = TOK // TCH                  # 8 chunks
CB = TCH // L                    # 4 batches per chunk
KT = D // 128                    # 4 contraction tiles
C3 = 3 * C                       # 165 unfolded conv rows


def _legalize_waits(nc, maxw=1):
    """This container's walrus rejects >1 sync-wait per instruction; move
    extra waits onto preceding same-engine NOPs."""
    cnt = [0]
    for f in nc.m.functions:
        for blk in f.blocks:
            newlist = []
            changed = False
            for ins in blk.instructions:
                si = getattr(ins, "sync_info", None)
                if si is not None and si.on_wait and len(si.on_wait) > maxw:
                    waits = list(si.on_wait)
                    extra, keep = waits[:-maxw], waits[-maxw:]
                    for i in range(0, len(extra), maxw):
                        cnt[0] += 1
                        nop = mybir.InstNoOp(
                            name=f"I-ws-{cnt[0]}", ins=[], outs=[], engine=ins.engine
                        )
                        nop.sync_info = mybir.SyncInfo(
                            on_wait=extra[i:i + maxw], on_update=[]
                        )
                        newlist.append(nop)
                    ins.sync_info = mybir.SyncInfo(
                        on_wait=keep, on_update=list(si.on_update)
                    )
                    changed = True
                newlist.append(ins)
            if changed:
                blk.instructions = newlist
    return nc


def _offset_ap(ap, extra_offset, dims):
    """AP at ap.offset + extra_offset (elements) with free dims `dims`
    ([[step, count], ...]), keeping ap's partition dim."""
    return bass.AP(tensor=ap.tensor, offset=ap.offset + extra_offset,
                   ap=[list(ap.ap[0])] + [list(d) for d in dims])


def build_nc():
    nc = bass.Bass()

    # ---- DRAM parameters (host-prepped) ----
    xcat0 = nc.declare_dram_parameter("xcat0", [128, TOK], F32R, isOutput=False)
    xcat1 = nc.declare_dram_parameter("xcat1", [C3 - 128, TOK], F32R, isOutput=False)
    wemb0 = nc.declare_dram_parameter("wemb0", [128, D], F32R, isOutput=False)
    wemb1 = nc.declare_dram_parameter("wemb1", [C3 - 128, D], F32R, isOutput=False)
    pe_d = nc.declare_dram_parameter("pe", [128, KT, L], F32, isOutput=False)
    wq_d = nc.declare_dram_parameter("wq", [128, NL, KT, D], F32R, isOutput=False)
    wk_d = nc.declare_dram_parameter("wk", [128, NL, KT, D], F32R, isOutput=False)
    wv_d = nc.declare_dram_parameter("wv", [128, NL, KT, D], F32R, isOutput=False)
    wo_d = nc.declare_dram_parameter("wo", [128, NL, KT, D], BF16, isOutput=False)
    w1_d = nc.declare_dram_parameter("w1", [128, NL, KT, DFF], F32R, isOutput=False)
    w2_d = nc.declare_dram_parameter("w2", [DFF, NL, KT, 128], F32R, isOutput=False)
    wout_d = nc.declare_dram_parameter("wout", [128, KT, C], F32R, isOutput=False)
    identb_d = nc.declare_dram_parameter("identb", [128, 128], BF16, isOutput=False)
    identr_d = nc.declare_dram_parameter("identr", [128, 128], F32R, isOutput=False)
    out_d = nc.declare_dram_parameter("out", [C, TOK], F32, isOutput=True)

    with tile.TileContext(nc) as tc, ExitStack() as stk:
        tc.race_detector_enabled = False
        singles = stk.enter_context(tc.tile_pool(name="singles", bufs=1))
        wp = stk.enter_context(tc.tile_pool(name="wp", bufs=2))
        xp = stk.enter_context(tc.tile_pool(name="xp", bufs=2))
        qp = stk.enter_context(tc.tile_pool(name="qp", bufs=2))
        kp = stk.enter_context(tc.tile_pool(name="kp", bufs=2))
        vp = stk.enter_context(tc.tile_pool(name="vp", bufs=2))
        expp = stk.enter_context(tc.tile_pool(name="expp", bufs=3))
        op_ = stk.enter_context(tc.tile_pool(name="op", bufs=3))
        rp = stk.enter_context(tc.tile_pool(name="rp", bufs=3))
        otp = stk.enter_context(tc.tile_pool(name="otp", bufs=2))
        yp = stk.enter_context(tc.tile_pool(name="yp", bufs=2))
        outp = stk.enter_context(tc.tile_pool(name="outp", bufs=2))
        # psum pools (8 banks total)
        pp = stk.enter_context(tc.tile_pool(name="pp", bufs=2, space="PSUM"))
        scp = stk.enter_context(tc.tile_pool(name="scp", bufs=2, space="PSUM"))
        oup = stk.enter_context(tc.tile_pool(name="oup", bufs=2, space="PSUM"))
        tpp = stk.enter_context(tc.tile_pool(name="tpp", bufs=2, space="PSUM"))

        # ---- persistent SBUF ----
        wemb0_sb = singles.tile([128, D], F32R)
        wemb1_sb = singles.tile([C3 - 128, D], F32R)
        pe_sb = singles.tile([128, KT, L], F32)
        wout_sb = singles.tile([128, KT, C], F32R)
        ident_b = singles.tile([128, 128], BF16)
        ident_r = singles.tile([128, 128], F32R)
        h_sb = [singles.tile([128, TOK], F32R, name=f"h{k}") for k in range(KT)]

        for dst, src in ((wemb0_sb, wemb0), (wemb1_sb, wemb1), (pe_sb, pe_d),
                         (wout_sb, wout_d), (ident_b, identb_d),
                         (ident_r, identr_d)):
            nc.sync.dma_start(out=dst[:], in_=src[:])

        # ---- token embedding: circular conv as matmul, + positional emb ----
        for t in range(NT):
            tsl = slice(t * TCH, (t + 1) * TCH)
            xc0 = xp.tile([128, TCH], F32R, tag="xc0")
            xc1 = xp.tile([C3 - 128, TCH], F32R, tag="xc1")
            nc.sync.dma_start(out=xc0[:], in_=xcat0[:, tsl])
            nc.sync.dma_start(out=xc1[:], in_=xcat1[:, tsl])
            for m in range(KT):
                ps = pp.tile([128, 512], F32, tag="pp")
                nc.tensor.matmul(ps[:, :TCH], wemb0_sb[:, m * 128:(m + 1) * 128],
                                 xc0[:], start=True, stop=False)
                nc.tensor.matmul(ps[:, :TCH], wemb1_sb[:, m * 128:(m + 1) * 128],
                                 xc1[:], start=False, stop=True)
                pe_b = _offset_ap(pe_sb[:, m, :], 0, [[0, CB], [1, L]])
                nc.vector.tensor_tensor(
                    h_sb[m][:, tsl].rearrange("p (b x) -> p b x", x=L),
                    ps[:, :TCH].rearrange("p (b x) -> p b x", x=L),
                    pe_b, op=OP.add)

        # ---- transformer layers ----
        for l in range(NL):
            wq_l = wp.tile([128, KT, D], F32R, tag="wq")
            wk_l = wp.tile([128, KT, D], F32R, tag="wk")
            wv_l = wp.tile([128, KT, D], F32R, tag="wv")
            wo_l = wp.tile([128, KT, D], BF16, tag="wo")
            w1_l = wp.tile([128, KT, DFF], F32R, tag="w1")
            w2_l = wp.tile([DFF, KT, 128], F32R, tag="w2")
            for dst, src in ((wq_l, wq_d), (wk_l, wk_d), (wv_l, wv_d),
                             (wo_l, wo_d), (w1_l, w1_d), (w2_l, w2_d)):
                nc.sync.dma_start(out=dst[:], in_=src[:, l])

            for g in range(NT):
                gsl = slice(g * TCH, (g + 1) * TCH)
                # Q/K projections for this chunk, feature-major [D, TCH]
                qc = [qp.tile([128, TCH], F32R, name=f"qc{m}", tag=f"qc{m}")
                      for m in range(KT)]
                kc = [kp.tile([128, TCH], F32R, name=f"kc{m}", tag=f"kc{m}")
                      for m in range(KT)]
                for m in range(KT):
                    msl = slice(m * 128, (m + 1) * 128)
                    ps = pp.tile([128, 512], F32, tag="pp")
                    for k in range(KT):
                        nc.tensor.matmul(ps[:, :TCH], wq_l[:, k, msl],
                                         h_sb[k][:, gsl],
                                         start=(k == 0), stop=(k == KT - 1))
                    nc.vector.tensor_copy(qc[m][:], ps[:, :TCH])
                for m in range(KT):
                    msl = slice(m * 128, (m + 1) * 128)
                    ps = pp.tile([128, 512], F32, tag="pp")
                    for k in range(KT):
                        nc.tensor.matmul(ps[:, :TCH], wk_l[:, k, msl],
                                         h_sb[k][:, gsl],
                                         start=(k == 0), stop=(k == KT - 1))
                    nc.vector.tensor_copy(kc[m][:], ps[:, :TCH])

                # V projection, token-major per batch (65-stride heads,
                # col 65h+64 = 1 for the softmax-sum trick)
                v_t = vp.tile([128, CB, 8 * 65], BF16, tag="v")
                nc.vector.memset(
                    v_t[:L, :, :].rearrange(
                        "p b (h x) -> p b h x", x=65)[:, :, :, 64:65], 1.0)
                for bi in range(CB):
                    b = g * CB + bi
                    bsl = slice(b * L, (b + 1) * L)
                    ps = pp.tile([128, 512], F32, tag="pp")
                    for k in range(KT):
                        nc.tensor.matmul(ps[:L, :], h_sb[k][:, bsl], wv_l[:, k, :],
                                         start=(k == 0), stop=(k == KT - 1))
                    nc.vector.tensor_copy(
                        v_t[:L, bi, :].rearrange("p (h x) -> p h x", x=65)[:, :, :64],
                        ps[:L, :].rearrange("p (h x) -> p h x", x=64))

                ot_t = [otp.tile([128, TCH], BF16, name=f"ot{m}", tag=f"ot{m}")
                        for m in range(KT)]
                for bi in range(CB):
                    csl = slice(bi * L, (bi + 1) * L)
                    # scoresT for 8 heads: even heads -> scA, odd -> scB
                    # (different PE row groups must write different PSUM banks)
                    scA = scp.tile([128, 512], F32, tag="sc")
                    scB = scp.tile([128, 512], F32, tag="sc")
                    for hh in range(8):
                        kt_i, base = divmod(hh * DK, 128)
                        sc = scA if hh % 2 == 0 else scB
                        col = (hh // 2) * 128
                        nc.tensor.matmul(sc[:L, col:col + L],
                                         kc[kt_i][base:base + DK, csl],
                                         qc[kt_i][base:base + DK, csl],
                                         start=True, stop=True)
                    exp_t = expp.tile([128, 8 * L], BF16, tag="exp")
                    # exp; head hh lands at exp_t cols hh*L
                    nc.scalar.activation(
                        exp_t[:L, :].rearrange("p (h x) -> p h x", x=2 * L)[:, :, :L],
                        scA[:L, :].rearrange("p (h x) -> p h x", x=128)[:, :, :L],
                        AF.Exp)
                    nc.scalar.activation(
                        _offset_ap(exp_t[:L, :], L, [[2 * L, 4], [1, L]]),
                        scB[:L, :].rearrange("p (h x) -> p h x", x=128)[:, :, :L],
                        AF.Exp)
                    # oU = expST.T @ [v | 1]  (token-major, col 64 = sum(exp))
                    ouA = oup.tile([128, 512], F32, tag="ou")
                    ouB = oup.tile([128, 512], F32, tag="ou")
                    for hh in range(8):
                        ou = ouA if hh < 4 else ouB
                        col = (hh % 4) * 128
                        nc.tensor.matmul(ou[:L, col:col + 65],
                                         exp_t[:L, hh * L:(hh + 1) * L],
                                         v_t[:L, bi, hh * 65:(hh + 1) * 65],
                                         start=True, stop=True)
                    r_t = rp.tile([128, 8], F32, tag="r")
                    o_t = op_.tile([128, D], BF16, tag="o")
                    for i, ou in enumerate((ouA, ouB)):
                        nc.vector.reciprocal(
                            r_t[:L, i * 4:(i + 1) * 4],
                            ou[:L, :].rearrange(
                                "p (h x) -> p h x", x=128)[:, :, 64:65])
                        nc.vector.tensor_tensor(
                            o_t[:L, i * 256:(i + 1) * 256].rearrange(
                                "p (h x) -> p h x", x=64),
                            ou[:L, :].rearrange(
                                "p (h x) -> p h x", x=128)[:, :, :64],
                            r_t[:L, i * 4:(i + 1) * 4].rearrange(
                                "p (h x) -> p h x", x=1).broadcast_to([L, 4, 64]),
                            op=OP.mult)
                    # transpose o back to feature-major
                    for m in range(KT):
                        tp = tpp.tile([128, 128], BF16, tag="tp")
                        nc.tensor.transpose(tp[:, :L],
                                            o_t[:L, m * 128:(m + 1) * 128],
                                            ident_b[:L, :L])
                        nc.scalar.copy(ot_t[m][:, csl], tp[:, :L])
                # Wo projection + residual (identity matmul accumulates h)
                for m in range(KT):
                    msl = slice(m * 128, (m + 1) * 128)
                    ps = pp.tile([128, 512], F32, tag="pp")
                    for k in range(KT):
                        nc.tensor.matmul(ps[:, :TCH], wo_l[:, k, msl], ot_t[k][:],
                                         start=(k == 0), stop=False)
                    nc.tensor.matmul(ps[:, :TCH], ident_r[:], h_sb[m][:, gsl],
                                     start=False, stop=True)
                    nc.scalar.copy(h_sb[m][:, gsl], ps[:, :TCH])
                # FFN + residual
                ps1 = pp.tile([128, 512], F32, tag="pp")
                for k in range(KT):
                    nc.tensor.matmul(ps1[:DFF, :TCH], w1_l[:, k, :],
                                     h_sb[k][:, gsl],
                                     start=(k == 0), stop=(k == KT - 1))
                y_t = yp.tile([DFF, TCH], F32R, tag="y")
                nc.scalar.activation(y_t[:, :], ps1[:DFF, :TCH], AF.Gelu)
                for m in range(KT):
                    ps2 = pp.tile([128, 512], F32, tag="pp")
                    nc.tensor.matmul(ps2[:, :TCH], w2_l[:, m, :], y_t[:, :],
                                     start=True, stop=False)
                    nc.tensor.matmul(ps2[:, :TCH], ident_r[:], h_sb[m][:, gsl],
                                     start=False, stop=True)
                    nc.scalar.copy(h_sb[m][:, gsl], ps2[:, :TCH])

        # ---- output projection [C, TOK] ----
        for t in range(NT):
            tsl = slice(t * TCH, (t + 1) * TCH)
            ps = pp.tile([128, 512], F32, tag="pp")
            for k in range(KT):
                nc.tensor.matmul(ps[:C, :TCH], wout_sb[:, k, :], h_sb[k][:, tsl],
                                 start=(k == 0), stop=(k == KT - 1))
            o_f = outp.tile([128, TCH], F32, tag="outc")
            nc.vector.tensor_copy(o_f[:C, :], ps[:C, :TCH])
            nc.sync.dma_start(out=out_d[:, tsl], in_=o_f[:C, :])

    return _legalize_waits(nc)


def _bf(a):
    return np.ascontiguousarray(a).astype(ml_dtypes.bfloat16)


def _r32(a):
    """Round to the f32r (tf32-like) grid: 10 explicit mantissa bits."""
    a = np.ascontiguousarray(a, np.float32)
    u = a.view(np.uint32).copy()
    u = (u + 0x1000) & 0xFFFFE000
    return u.view(np.float32)


def prep_weights(tok_w, pe, Wq, Wk, Wv, Wo, W1, W2, proj_w):
    """Host-side weight reorganization (shared across cores)."""
    scale = 1.0 / math.sqrt(DK)
    # conv unfold: W_unf[55d + c, o] = tok_w[o, c, d]
    wemb = np.ascontiguousarray(np.transpose(tok_w, (2, 1, 0))).reshape(C3, D)
    # projection weights as lhsT tiles: w[p, l, k, j] = W[l, j, 128k + p]
    def proj_lhsT(W):  # [NL, D_out, D_in] -> [128, NL, KT, D_out]
        return np.ascontiguousarray(
            np.transpose(W, (2, 0, 1)).reshape(KT, 128, NL, W.shape[1])
            .transpose(1, 2, 0, 3))
    eye = np.eye(128, dtype=np.float32)
    m = {
        "identb": _bf(eye), "identr": eye.copy(),
        "wemb0": _r32(wemb[:128]), "wemb1": _r32(wemb[128:]),
        "pe": np.ascontiguousarray(
            np.ascontiguousarray(pe.T).reshape(KT, 128, L).transpose(1, 0, 2)),
        "wq": _r32(proj_lhsT(Wq * scale)),
        "wk": _r32(proj_lhsT(Wk)),
        "wv": _r32(proj_lhsT(Wv)),
        "wo": _bf(proj_lhsT(Wo)),
        "w1": _r32(proj_lhsT(W1)),
        # w2[p, l, m, j] = W2[l, 128m + j, p]   (p over DFF=64)
        "w2": _r32(np.transpose(W2, (2, 0, 1)).reshape(DFF, NL, KT, 128)),
        # wout[p, k, j] = proj_w[j, 128k + p]
        "wout": _r32(np.ascontiguousarray(proj_w.T).reshape(KT, 128, C)
                     .transpose(1, 0, 2)),
    }
    return m


def prep_xcat(xs):
    """Per-core input: xs [BL, L, C] -> circular-unfolded feature-major
    [165, BL*L], split into [128, .] + [37, .]."""
    xt = np.ascontiguousarray(np.transpose(xs, (2, 0, 1)))    # [C, BL, L]
    rows = [np.roll(xt, 1 - d, axis=2) for d in range(3)]     # x[t+d-1]
    xcat = np.concatenate(rows, axis=0).reshape(C3, TOK)
    return _r32(xcat[:128]), _r32(xcat[128:])


_NC_CACHE = {}


def get_nc():
    if "nc" not in _NC_CACHE:
        _NC_CACHE["nc"] = build_nc()
    return _NC_CACHE["nc"]


def make_in_maps(inputs):
    x = np.asarray(inputs["x"], np.float32)
    wm = prep_weights(np.asarray(inputs["tok_w"], np.float32),
                      np.asarray(inputs["pe"], np.float32),
                      np.asarray(inputs["Wq"], np.float32),
                      np.asarray(inputs["Wk"], np.float32),
                      np.asarray(inputs["Wv"], np.float32),
                      np.asarray(inputs["Wo"], np.float32),
                      np.asarray(inputs["W1"], np.float32),
                      np.asarray(inputs["W2"], np.float32),
                      np.asarray(inputs["proj_w"], np.float32))
    in_maps = []
    for c in range(NCORES):
        x0, x1 = prep_xcat(x[c * BL:(c + 1) * BL])
        in_maps.append({**wm, "xcat0": x0, "xcat1": x1})
    return in_maps


def assemble_out(results):
    # per-core out [C, TOK] feature-major -> [B, L, C]
    outs = [np.asarray(r["out"], np.float32).reshape(C, BL, L).transpose(1, 2, 0)
            for r in results]
    return np.concatenate(outs, axis=0)


def kernel(**inputs) -> np.ndarray:
    nc = get_nc()
    in_maps = make_in_maps(inputs)
    res = run_bass_kernel_spmd(nc, in_maps, core_ids=list(range(NCORES)))
    return assemble_out(res.results)
